# revision 1
# baseline (speedup 1.0000x reference)
"""DMPNN encoder on 8 Trainium2 NeuronCores (Bass/Tile, SPMD).

Strategy: shard undirected edge pairs across cores (reverse edges stay
local). Each core sorts its edges by dst into a padded layout (392
node-blocks x 384 edge capacity). Message-passing iteration k:
  h_{k-1} = relu(h0 + pW2_k[src] - hW2_{k-2}[rev])
assembled per 128-edge tile from sequential h0, an indirect row gather of
the node table, and a sequential read of the rev-scattered hW2 buffer.
Segment-sum is a one-hot matmul accumulated in PSUM per node block.
Node partials are ReduceScattered; pW2 slices are AllGathered.
x@W1 and x@W3x are hoisted to node space (no per-edge transposes of x).
"""
import sys, os
sys.path.insert(0, "/opt/trn_rl_repo")
import numpy as np

N = 50000
E = 800000
H = 128
NC = 8
ELOC = E // NC            # 100000
NBLK = 392
NPAD = NBLK * 128         # 50176
TPB = 3
CBLK = TPB * 128          # 384
T = NBLK * TPB            # 1176 tiles of 128 edges
EPAD = T * 128            # 150528
NSLICE = NBLK // NC       # 49 blocks per core slice
NG = 512

_prog = None
LAST_EXEC_NS = None


def _build_program():
    global _prog
    if _prog is not None:
        return _prog
    import concourse.bass as bass
    import concourse.mybir as mybir
    import concourse.tile as tile
    from concourse import bacc
    from concourse.masks import make_identity
    from contextlib import ExitStack

    f32 = mybir.dt.float32
    i32 = mybir.dt.int32

    nc = bacc.Bacc("TRN2", target_bir_lowering=False, debug=False, num_devices=NC)

    def inp(name, shape):
        return nc.dram_tensor(name, shape, f32, kind="ExternalInput").ap()

    xT   = inp("xT",   [133, NPAD])
    xsT  = inp("xsT",  [133, NSLICE * 128])
    eaT  = inp("eaT",  [14, EPAD])
    S    = inp("S",    [EPAD, 128])
    GB   = inp("GB",   [NSLICE, 128, NG])
    W1x1 = inp("W1x1", [128, 128])
    W1x2 = inp("W1x2", [5, 128])
    W1e  = inp("W1e",  [14, 128])
    W2   = inp("W2",   [128, 128])
    W3x1 = inp("W3x1", [128, 128])
    W3x2 = inp("W3x2", [5, 128])
    W3v  = inp("W3v",  [128, 128])
    srcT = nc.dram_tensor("srcT", [128, T], i32, kind="ExternalInput").ap()
    revT = nc.dram_tensor("revT", [128, T], i32, kind="ExternalInput").ap()
    outp = nc.dram_tensor("outp", [NG, H], f32, kind="ExternalOutput").ap()

    XW    = nc.dram_tensor("XW",    [NPAD, H], f32).ap()
    XW3   = nc.dram_tensor("XW3",   [NSLICE, 128, H], f32).ap()
    h0d   = nc.dram_tensor("h0d",   [EPAD, H], f32).ap()
    HRA   = nc.dram_tensor("HRA",   [EPAD, H], f32).ap()
    HRB   = nc.dram_tensor("HRB",   [EPAD, H], f32).ap()
    ndin  = nc.dram_tensor("ndin",  [NBLK, 128, H], f32).ap()
    nsl   = nc.dram_tensor("nsl",   [NSLICE, 128, H], f32).ap()
    pw2s  = nc.dram_tensor("pw2s",  [NSLICE * 128, H], f32).ap()
    pw2f  = nc.dram_tensor("pw2f",  [NPAD, H], f32, addr_space="Shared").ap()

    groups = [list(range(NC))]

    with tile.TileContext(nc) as tc, ExitStack() as ctx:
        consts = ctx.enter_context(tc.tile_pool(name="consts", bufs=1))
        sb = ctx.enter_context(tc.tile_pool(name="sb", bufs=3))
        ps_main = ctx.enter_context(tc.tile_pool(name="ps_main", bufs=2, space="PSUM"))
        ps = ps_main

        ident = consts.tile([128, 128], f32)
        make_identity(nc, ident[:])

        def const_tile(src_ap, shape, cname):
            t_ = consts.tile(shape, f32, name=cname, tag=cname)
            nc.sync.dma_start(out=t_[:], in_=src_ap[:])
            return t_

        w1x1 = const_tile(W1x1, [128, 128], "w1x1")
        w1x2 = const_tile(W1x2, [5, 128], "w1x2")
        w1e  = const_tile(W1e,  [14, 128], "w1e")
        w2   = const_tile(W2,   [128, 128], "w2")
        w3x1 = const_tile(W3x1, [128, 128], "w3x1")
        w3x2 = const_tile(W3x2, [5, 128], "w3x2")
        w3v  = const_tile(W3v,  [128, 128], "w3v")
        sidx = consts.tile([128, T], i32)
        nc.sync.dma_start(out=sidx[:], in_=srcT[:])
        ridx = consts.tile([128, T], i32)
        nc.sync.dma_start(out=ridx[:], in_=revT[:])

        # ---- PRE: XW = x @ W1x (all blocks), XW3 = x_slice @ W3x (own slice)
        for b in range(NBLK):
            cols = slice(b * 128, (b + 1) * 128)
            xt1 = sb.tile([128, 128], f32, tag="xt1")
            nc.sync.dma_start(out=xt1[:], in_=xT[0:128, cols])
            xt2 = sb.tile([5, 128], f32, tag="xt2")
            nc.sync.dma_start(out=xt2[:], in_=xT[128:133, cols])
            pw = ps.tile([128, 128], f32, tag="psw")
            nc.tensor.matmul(out=pw[:], lhsT=xt1[:], rhs=w1x1[:], start=True, stop=False)
            nc.tensor.matmul(out=pw[:], lhsT=xt2[:], rhs=w1x2[:], start=False, stop=True)
            xwb = sb.tile([128, 128], f32, tag="xwb")
            nc.vector.tensor_copy(out=xwb[:], in_=pw[:])
            nc.scalar.dma_start(out=XW[b * 128:(b + 1) * 128, :], in_=xwb[:])
        for b in range(NSLICE):
            cols = slice(b * 128, (b + 1) * 128)
            xt1 = sb.tile([128, 128], f32, tag="xt1")
            nc.sync.dma_start(out=xt1[:], in_=xsT[0:128, cols])
            xt2 = sb.tile([5, 128], f32, tag="xt2")
            nc.sync.dma_start(out=xt2[:], in_=xsT[128:133, cols])
            pw = ps.tile([128, 128], f32, tag="psw")
            nc.tensor.matmul(out=pw[:], lhsT=xt1[:], rhs=w3x1[:], start=True, stop=False)
            nc.tensor.matmul(out=pw[:], lhsT=xt2[:], rhs=w3x2[:], start=False, stop=True)
            xwb = sb.tile([128, 128], f32, tag="xwb")
            nc.vector.tensor_copy(out=xwb[:], in_=pw[:])
            nc.scalar.dma_start(out=XW3[b], in_=xwb[:])

        # ---- edge sweeps
        def sweep(k):
            hr_rd = HRA if k == 2 else HRB
            hr_wr = HRA if k == 1 else HRB
            for b in range(NBLK):
                pnode = ps.tile([128, 128], f32, tag="node")
                for j in range(TPB):
                    t = b * TPB + j
                    rows = slice(t * 128, (t + 1) * 128)
                    if k == 1:
                        g = sb.tile([128, 128], f32, tag="g")
                        nc.gpsimd.indirect_dma_start(
                            out=g[:], out_offset=None, in_=XW[:],
                            in_offset=bass.IndirectOffsetOnAxis(ap=sidx[:, t:t + 1], axis=0))
                        eat = sb.tile([14, 128], f32, tag="eat")
                        nc.sync.dma_start(out=eat[:], in_=eaT[:, rows.start:rows.stop])
                        pe = ps.tile([128, 128], f32, tag="pse")
                        nc.tensor.matmul(out=pe[:], lhsT=eat[:], rhs=w1e[:], start=True, stop=True)
                        t1 = sb.tile([128, 128], f32, tag="t1")
                        nc.vector.tensor_add(out=t1[:], in0=g[:], in1=pe[:])
                        h = sb.tile([128, 128], f32, tag="h")
                        nc.vector.tensor_relu(out=h[:], in_=t1[:])
                        nc.scalar.dma_start(out=h0d[rows, :], in_=h[:])
                    else:
                        g = sb.tile([128, 128], f32, tag="g")
                        nc.gpsimd.indirect_dma_start(
                            out=g[:], out_offset=None, in_=pw2f[:],
                            in_offset=bass.IndirectOffsetOnAxis(ap=sidx[:, t:t + 1], axis=0))
                        h0t = sb.tile([128, 128], f32, tag="h0t")
                        nc.sync.dma_start(out=h0t[:], in_=h0d[rows, :])
                        hrt = sb.tile([128, 128], f32, tag="hrt")
                        nc.sync.dma_start(out=hrt[:], in_=hr_rd[rows, :])
                        t1 = sb.tile([128, 128], f32, tag="t1")
                        nc.vector.tensor_sub(out=t1[:], in0=g[:], in1=hrt[:])
                        t2 = sb.tile([128, 128], f32, tag="t2")
                        nc.vector.tensor_add(out=t2[:], in0=t1[:], in1=h0t[:])
                        h = sb.tile([128, 128], f32, tag="h")
                        nc.vector.tensor_relu(out=h[:], in_=t2[:])
                    st = sb.tile([128, 128], f32, tag="St")
                    nc.sync.dma_start(out=st[:], in_=S[rows, :])
                    nc.tensor.matmul(out=pnode[:], lhsT=st[:], rhs=h[:],
                                     start=(j == 0), stop=(j == TPB - 1))
                    if k < 3:
                        pT = ps.tile([128, 128], f32, tag="psT")
                        nc.tensor.transpose(out=pT[:], in_=h[:], identity=ident[:])
                        hT = sb.tile([128, 128], f32, tag="hT")
                        nc.vector.tensor_copy(out=hT[:], in_=pT[:])
                        pw = ps.tile([128, 128], f32, tag="psw")
                        nc.tensor.matmul(out=pw[:], lhsT=hT[:], rhs=w2[:], start=True, stop=True)
                        hw = sb.tile([128, 128], f32, tag="hw")
                        nc.vector.tensor_copy(out=hw[:], in_=pw[:])
                        nc.gpsimd.indirect_dma_start(
                            out=hr_wr[:],
                            out_offset=bass.IndirectOffsetOnAxis(ap=ridx[:, t:t + 1], axis=0),
                            in_=hw[:], in_offset=None)
                nb = sb.tile([128, 128], f32, tag="nb")
                nc.vector.tensor_copy(out=nb[:], in_=pnode[:])
                nc.scalar.dma_start(out=ndin[b], in_=nb[:])

        def collective(k):
            nc.gpsimd.collective_compute(
                "ReduceScatter", mybir.AluOpType.add, replica_groups=groups,
                ins=[ndin[:]], outs=[nsl[:]])
            if k < 3:
                for b in range(NSLICE):
                    nsb = sb.tile([128, 128], f32, tag="nsb")
                    nc.sync.dma_start(out=nsb[:], in_=nsl[b])
                    pT = ps.tile([128, 128], f32, tag="psT")
                    nc.tensor.transpose(out=pT[:], in_=nsb[:], identity=ident[:])
                    nT = sb.tile([128, 128], f32, tag="hT")
                    nc.vector.tensor_copy(out=nT[:], in_=pT[:])
                    pw = ps.tile([128, 128], f32, tag="psw")
                    nc.tensor.matmul(out=pw[:], lhsT=nT[:], rhs=w2[:], start=True, stop=True)
                    pb = sb.tile([128, 128], f32, tag="hw")
                    nc.vector.tensor_copy(out=pb[:], in_=pw[:])
                    nc.scalar.dma_start(out=pw2s[b * 128:(b + 1) * 128, :], in_=pb[:])
                nc.gpsimd.collective_compute(
                    "AllGather", mybir.AluOpType.bypass, replica_groups=groups,
                    ins=[pw2s[:]], outs=[pw2f[:]])

        sweep(1)
        collective(1)
        sweep(2)
        collective(2)
        sweep(3)
        collective(3)

        # ---- final: node_attr = relu(XW3 + vmsg @ W3v); out = GB^T @ node_attr
        out_acc = consts.tile([128, 4 * 128], f32, name="out_acc")
        nc.vector.memset(out_acc[:], 0.0)
        for b in range(NSLICE):
            vb = sb.tile([128, 128], f32, tag="nsb")
            nc.sync.dma_start(out=vb[:], in_=nsl[b])
            pT = ps.tile([128, 128], f32, tag="psT")
            nc.tensor.transpose(out=pT[:], in_=vb[:], identity=ident[:])
            vT = sb.tile([128, 128], f32, tag="hT")
            nc.vector.tensor_copy(out=vT[:], in_=pT[:])
            pn = ps.tile([128, 128], f32, tag="pse")
            nc.tensor.matmul(out=pn[:], lhsT=vT[:], rhs=w3v[:], start=True, stop=True)
            x3b = sb.tile([128, 128], f32, tag="h0t")
            nc.sync.dma_start(out=x3b[:], in_=XW3[b])
            t1 = sb.tile([128, 128], f32, tag="t1")
            nc.vector.tensor_add(out=t1[:], in0=x3b[:], in1=pn[:])
            na = sb.tile([128, 128], f32, tag="h")
            nc.vector.tensor_relu(out=na[:], in_=t1[:])
            gb = sb.tile([128, NG], f32, tag="gb")
            nc.sync.dma_start(out=gb[:], in_=GB[b])
            for g4 in range(4):
                po = ps.tile([128, 128], f32, tag="psw", name="po")
                nc.tensor.matmul(out=po[:], lhsT=gb[:, g4 * 128:(g4 + 1) * 128],
                                 rhs=na[:], start=True, stop=True)
                gsl = slice(g4 * 128, (g4 + 1) * 128)
                nc.vector.tensor_add(out=out_acc[:, gsl], in0=out_acc[:, gsl], in1=po[:])
        for g4 in range(4):
            nc.scalar.dma_start(out=outp[g4 * 128:(g4 + 1) * 128, :],
                                in_=out_acc[:, g4 * 128:(g4 + 1) * 128])

    nc.compile()
    _prog = nc
    return nc


def _host_layout(x, edge_attr, edge_index, batch):
    src_all = np.asarray(edge_index[0]).astype(np.int64)
    dst_all = np.asarray(edge_index[1]).astype(np.int64)
    batch = np.asarray(batch).astype(np.int64)

    xTfull = np.zeros((133, NPAD), np.float32)
    xTfull[:, :N] = np.asarray(x, np.float32).T

    per_core = []
    for c in range(NC):
        lo = c * ELOC
        src = src_all[lo:lo + ELOC]
        dst = dst_all[lo:lo + ELOC]
        order = np.argsort(dst, kind="stable")
        dsts = dst[order]
        blk = dsts >> 7
        cnt = np.bincount(blk, minlength=NBLK)
        assert cnt.max() <= CBLK, f"block overflow {cnt.max()}"
        start = np.zeros(NBLK, np.int64)
        start[1:] = np.cumsum(cnt)[:-1]
        rank = np.arange(ELOC) - start[blk]
        pos_sorted = blk * CBLK + rank
        posmap = np.empty(ELOC, np.int64)
        posmap[order] = pos_sorted

        src_pad = np.zeros(EPAD, np.int32)
        src_pad[pos_sorted] = src[order].astype(np.int32)
        rev_pad = np.arange(EPAD, dtype=np.int32)
        rev_pad[posmap] = posmap[np.arange(ELOC) ^ 1].astype(np.int32)

        Sc = np.zeros((EPAD, 128), np.float32)
        Sc[pos_sorted, (dsts & 127)] = 1.0

        eaTc = np.zeros((14, EPAD), np.float32)
        eaTc[:, pos_sorted] = np.asarray(edge_attr[lo:lo + ELOC], np.float32)[order].T

        nlo = c * NSLICE * 128
        gb_flat = np.zeros((NSLICE * 128, NG), np.float32)
        nodes = np.arange(nlo, min(nlo + NSLICE * 128, N))
        gb_flat[nodes - nlo, batch[nodes]] = 1.0

        per_core.append(dict(
            eaT=np.ascontiguousarray(eaTc),
            S=Sc,
            srcT=np.ascontiguousarray(src_pad.reshape(T, 128).T),
            revT=np.ascontiguousarray(rev_pad.reshape(T, 128).T),
            GB=np.ascontiguousarray(gb_flat.reshape(NSLICE, 128, NG)),
            xsT=np.ascontiguousarray(xTfull[:, nlo:nlo + NSLICE * 128]),
        ))
    return xTfull, per_core


def kernel(x, edge_attr, W1, W2, W3, edge_index, rev_index, batch):
    global LAST_EXEC_NS
    from concourse.bass_utils import run_bass_kernel_spmd

    x = np.asarray(x, np.float32)
    edge_attr = np.asarray(edge_attr, np.float32)
    W1 = np.asarray(W1, np.float32)
    W2m = np.asarray(W2, np.float32)
    W3 = np.asarray(W3, np.float32)

    nc = _build_program()
    xTfull, per_core = _host_layout(x, edge_attr, edge_index, batch)

    shared = dict(
        xT=xTfull,
        W1x1=np.ascontiguousarray(W1[0:128]),
        W1x2=np.ascontiguousarray(W1[128:133]),
        W1e=np.ascontiguousarray(W1[133:147]),
        W2=W2m,
        W3x1=np.ascontiguousarray(W3[0:128]),
        W3x2=np.ascontiguousarray(W3[128:133]),
        W3v=np.ascontiguousarray(W3[133:261]),
    )
    in_maps = [{**shared, **pc} for pc in per_core]

    trace = os.environ.get("BASS_KERNEL_TRACE", "0") == "1"
    import time as _time
    t0 = _time.time()
    res = run_bass_kernel_spmd(nc, in_maps, list(range(NC)), trace=trace)
    t1 = _time.time()
    LAST_EXEC_NS = res.exec_time_ns
    if LAST_EXEC_NS is None:
        LAST_EXEC_NS = int((t1 - t0) * 1e9)  # wall-clock fallback (incl. upload)

    out = np.zeros((NG, H), np.float32)
    for c in range(NC):
        out += res.results[c]["outp"]
    return out



# revision 2
# speedup vs baseline: 2.3980x; 2.3980x over previous
"""DMPNN encoder on 8 Trainium2 NeuronCores (Bass/Tile, SPMD).

Strategy: shard undirected edge pairs across cores (reverse edges stay
local). Each core sorts its edges by dst into a padded layout (392
node-blocks x 384 edge capacity). Message-passing iteration k:
  h_{k-1} = relu(h0 + pW2_k[src] - hW2_{k-2}[rev])
assembled per 128-edge tile from sequential h0, an indirect row gather of
the node table, and a sequential read of the rev-scattered hW2 buffer.
Segment-sum is a one-hot matmul accumulated in PSUM per node block; the
one-hot tiles are built on device (iota vs dst&127 compare) instead of
being uploaded. Node features are uploaded as per-core slices only and
x@W1x is AllGathered on device. Node partials are ReduceScattered; pW2
slices are AllGathered. The graph-pooling one-hot is also built on
device from batch ids.
"""
import sys, os
sys.path.insert(0, "/opt/trn_rl_repo")
import numpy as np

N = 50000
E = 800000
H = 128
NC = 8
ELOC = E // NC            # 100000
NBLK = 392
NPAD = NBLK * 128         # 50176
TPB = 3
CBLK = TPB * 128          # 384
T = NBLK * TPB            # 1176 tiles of 128 edges
EPAD = T * 128            # 150528
NSLICE = NBLK // NC       # 49 blocks per core slice
NG = 512

_prog = None
LAST_EXEC_NS = None


def _build_program():
    global _prog
    if _prog is not None:
        return _prog
    import concourse.bass as bass
    import concourse.mybir as mybir
    import concourse.tile as tile
    from concourse import bacc
    from concourse.masks import make_identity
    from contextlib import ExitStack

    f32 = mybir.dt.float32
    i32 = mybir.dt.int32

    nc = bacc.Bacc("TRN2", target_bir_lowering=False, debug=False, num_devices=NC)

    def inp(name, shape, dt=f32):
        return nc.dram_tensor(name, shape, dt, kind="ExternalInput").ap()

    xsT  = inp("xsT",  [133, NSLICE * 128])
    eaT  = inp("eaT",  [14, EPAD])
    W1x1 = inp("W1x1", [128, 128])
    W1x2 = inp("W1x2", [5, 128])
    W1e  = inp("W1e",  [14, 128])
    W2   = inp("W2",   [128, 128])
    W3x1 = inp("W3x1", [128, 128])
    W3x2 = inp("W3x2", [5, 128])
    W3v  = inp("W3v",  [128, 128])
    srcT = inp("srcT", [128, T], i32)
    revT = inp("revT", [128, T], i32)
    dstT = inp("dstT", [128, T])
    batT = inp("batT", [128, NSLICE])
    outp = nc.dram_tensor("outp", [NG, H], f32, kind="ExternalOutput").ap()

    XWs   = nc.dram_tensor("XWs",   [NSLICE * 128, H], f32).ap()
    XWf   = nc.dram_tensor("XWf",   [NPAD, H], f32, addr_space="Shared").ap()
    XW3   = nc.dram_tensor("XW3",   [NSLICE, 128, H], f32).ap()
    h0d   = nc.dram_tensor("h0d",   [EPAD, H], f32).ap()
    HRA   = nc.dram_tensor("HRA",   [EPAD, H], f32).ap()
    HRB   = nc.dram_tensor("HRB",   [EPAD, H], f32).ap()
    ndin  = nc.dram_tensor("ndin",  [NBLK, 128, H], f32).ap()
    nsl   = nc.dram_tensor("nsl",   [NSLICE, 128, H], f32).ap()
    pw2s  = nc.dram_tensor("pw2s",  [NSLICE * 128, H], f32).ap()
    pw2f  = nc.dram_tensor("pw2f",  [NPAD, H], f32, addr_space="Shared").ap()

    groups = [list(range(NC))]

    with tile.TileContext(nc) as tc, ExitStack() as ctx:
        consts = ctx.enter_context(tc.tile_pool(name="consts", bufs=1))
        sb = ctx.enter_context(tc.tile_pool(name="sb", bufs=3))
        ps_main = ctx.enter_context(tc.tile_pool(name="ps_main", bufs=2, space="PSUM"))
        ps = ps_main

        ident = consts.tile([128, 128], f32)
        make_identity(nc, ident[:])
        iota1 = consts.tile([128, 128], f32, name="iota1", tag="iota1")
        nc.gpsimd.iota(iota1[:], pattern=[[1, 128]], base=0, channel_multiplier=0,
                       allow_small_or_imprecise_dtypes=True)
        iota5 = consts.tile([128, NG], f32, name="iota5", tag="iota5")
        nc.gpsimd.iota(iota5[:], pattern=[[1, NG]], base=0, channel_multiplier=0,
                       allow_small_or_imprecise_dtypes=True)

        def const_tile(src_ap, shape, cname, dt=f32):
            t_ = consts.tile(shape, dt, name=cname, tag=cname)
            nc.sync.dma_start(out=t_[:], in_=src_ap[:])
            return t_

        w1x1 = const_tile(W1x1, [128, 128], "w1x1")
        w1x2 = const_tile(W1x2, [5, 128], "w1x2")
        w1e  = const_tile(W1e,  [14, 128], "w1e")
        w2   = const_tile(W2,   [128, 128], "w2")
        w3x1 = const_tile(W3x1, [128, 128], "w3x1")
        w3x2 = const_tile(W3x2, [5, 128], "w3x2")
        w3v  = const_tile(W3v,  [128, 128], "w3v")
        sidx = const_tile(srcT, [128, T], "sidx", i32)
        ridx = const_tile(revT, [128, T], "ridx", i32)
        didx = const_tile(dstT, [128, T], "didx")
        bidx = const_tile(batT, [128, NSLICE], "bidx")

        # ---- PRE: XWs = x_slice @ W1x, XW3 = x_slice @ W3x; AllGather XWs
        for b in range(NSLICE):
            cols = slice(b * 128, (b + 1) * 128)
            xt1 = sb.tile([128, 128], f32, tag="xt1")
            nc.sync.dma_start(out=xt1[:], in_=xsT[0:128, cols])
            xt2 = sb.tile([5, 128], f32, tag="xt2")
            nc.sync.dma_start(out=xt2[:], in_=xsT[128:133, cols])
            pw = ps.tile([128, 128], f32, tag="psw")
            nc.tensor.matmul(out=pw[:], lhsT=xt1[:], rhs=w1x1[:], start=True, stop=False)
            nc.tensor.matmul(out=pw[:], lhsT=xt2[:], rhs=w1x2[:], start=False, stop=True)
            xwb = sb.tile([128, 128], f32, tag="xwb")
            nc.vector.tensor_copy(out=xwb[:], in_=pw[:])
            nc.scalar.dma_start(out=XWs[b * 128:(b + 1) * 128, :], in_=xwb[:])
            pw3 = ps.tile([128, 128], f32, tag="psw")
            nc.tensor.matmul(out=pw3[:], lhsT=xt1[:], rhs=w3x1[:], start=True, stop=False)
            nc.tensor.matmul(out=pw3[:], lhsT=xt2[:], rhs=w3x2[:], start=False, stop=True)
            xwb3 = sb.tile([128, 128], f32, tag="xwb")
            nc.vector.tensor_copy(out=xwb3[:], in_=pw3[:])
            nc.scalar.dma_start(out=XW3[b], in_=xwb3[:])
        nc.gpsimd.collective_compute(
            "AllGather", mybir.AluOpType.bypass, replica_groups=groups,
            ins=[XWs[:]], outs=[XWf[:]])

        # ---- edge sweeps
        def sweep(k):
            hr_rd = HRA if k == 2 else HRB
            hr_wr = HRA if k == 1 else HRB
            for b in range(NBLK):
                pnode = ps.tile([128, 128], f32, tag="node")
                for j in range(TPB):
                    t = b * TPB + j
                    rows = slice(t * 128, (t + 1) * 128)
                    if k == 1:
                        g = sb.tile([128, 128], f32, tag="g")
                        nc.gpsimd.indirect_dma_start(
                            out=g[:], out_offset=None, in_=XWf[:],
                            in_offset=bass.IndirectOffsetOnAxis(ap=sidx[:, t:t + 1], axis=0))
                        eat = sb.tile([14, 128], f32, tag="eat")
                        nc.sync.dma_start(out=eat[:], in_=eaT[:, rows.start:rows.stop])
                        pe = ps.tile([128, 128], f32, tag="pse")
                        nc.tensor.matmul(out=pe[:], lhsT=eat[:], rhs=w1e[:], start=True, stop=True)
                        t1 = sb.tile([128, 128], f32, tag="t1")
                        nc.vector.tensor_add(out=t1[:], in0=g[:], in1=pe[:])
                        h = sb.tile([128, 128], f32, tag="h")
                        nc.vector.tensor_relu(out=h[:], in_=t1[:])
                        nc.scalar.dma_start(out=h0d[rows, :], in_=h[:])
                    else:
                        g = sb.tile([128, 128], f32, tag="g")
                        nc.gpsimd.indirect_dma_start(
                            out=g[:], out_offset=None, in_=pw2f[:],
                            in_offset=bass.IndirectOffsetOnAxis(ap=sidx[:, t:t + 1], axis=0))
                        h0t = sb.tile([128, 128], f32, tag="h0t")
                        nc.sync.dma_start(out=h0t[:], in_=h0d[rows, :])
                        hrt = sb.tile([128, 128], f32, tag="hrt")
                        nc.sync.dma_start(out=hrt[:], in_=hr_rd[rows, :])
                        t1 = sb.tile([128, 128], f32, tag="t1")
                        nc.vector.tensor_sub(out=t1[:], in0=g[:], in1=hrt[:])
                        t2 = sb.tile([128, 128], f32, tag="t2")
                        nc.vector.tensor_add(out=t2[:], in0=t1[:], in1=h0t[:])
                        h = sb.tile([128, 128], f32, tag="h")
                        nc.vector.tensor_relu(out=h[:], in_=t2[:])
                    st = sb.tile([128, 128], f32, tag="St")
                    nc.vector.tensor_scalar(
                        out=st[:], in0=iota1[:], scalar1=didx[:, t:t + 1],
                        scalar2=None, op0=mybir.AluOpType.is_equal)
                    nc.tensor.matmul(out=pnode[:], lhsT=st[:], rhs=h[:],
                                     start=(j == 0), stop=(j == TPB - 1))
                    if k < 3:
                        pT = ps.tile([128, 128], f32, tag="psT")
                        nc.tensor.transpose(out=pT[:], in_=h[:], identity=ident[:])
                        hT = sb.tile([128, 128], f32, tag="hT")
                        nc.vector.tensor_copy(out=hT[:], in_=pT[:])
                        pw = ps.tile([128, 128], f32, tag="psw")
                        nc.tensor.matmul(out=pw[:], lhsT=hT[:], rhs=w2[:], start=True, stop=True)
                        hw = sb.tile([128, 128], f32, tag="hw")
                        nc.vector.tensor_copy(out=hw[:], in_=pw[:])
                        nc.gpsimd.indirect_dma_start(
                            out=hr_wr[:],
                            out_offset=bass.IndirectOffsetOnAxis(ap=ridx[:, t:t + 1], axis=0),
                            in_=hw[:], in_offset=None)
                nb = sb.tile([128, 128], f32, tag="nb")
                nc.vector.tensor_copy(out=nb[:], in_=pnode[:])
                nc.scalar.dma_start(out=ndin[b], in_=nb[:])

        def collective(k):
            nc.gpsimd.collective_compute(
                "ReduceScatter", mybir.AluOpType.add, replica_groups=groups,
                ins=[ndin[:]], outs=[nsl[:]])
            if k < 3:
                for b in range(NSLICE):
                    nsb = sb.tile([128, 128], f32, tag="nsb")
                    nc.sync.dma_start(out=nsb[:], in_=nsl[b])
                    pT = ps.tile([128, 128], f32, tag="psT")
                    nc.tensor.transpose(out=pT[:], in_=nsb[:], identity=ident[:])
                    nT = sb.tile([128, 128], f32, tag="hT")
                    nc.vector.tensor_copy(out=nT[:], in_=pT[:])
                    pw = ps.tile([128, 128], f32, tag="psw")
                    nc.tensor.matmul(out=pw[:], lhsT=nT[:], rhs=w2[:], start=True, stop=True)
                    pb = sb.tile([128, 128], f32, tag="hw")
                    nc.vector.tensor_copy(out=pb[:], in_=pw[:])
                    nc.scalar.dma_start(out=pw2s[b * 128:(b + 1) * 128, :], in_=pb[:])
                nc.gpsimd.collective_compute(
                    "AllGather", mybir.AluOpType.bypass, replica_groups=groups,
                    ins=[pw2s[:]], outs=[pw2f[:]])

        sweep(1)
        collective(1)
        sweep(2)
        collective(2)
        sweep(3)
        collective(3)

        # ---- final: node_attr = relu(XW3 + vmsg @ W3v); out = GB^T @ node_attr
        out_acc = consts.tile([128, 4 * 128], f32, name="out_acc")
        nc.vector.memset(out_acc[:], 0.0)
        for b in range(NSLICE):
            vb = sb.tile([128, 128], f32, tag="nsb")
            nc.sync.dma_start(out=vb[:], in_=nsl[b])
            pT = ps.tile([128, 128], f32, tag="psT")
            nc.tensor.transpose(out=pT[:], in_=vb[:], identity=ident[:])
            vT = sb.tile([128, 128], f32, tag="hT")
            nc.vector.tensor_copy(out=vT[:], in_=pT[:])
            pn = ps.tile([128, 128], f32, tag="pse")
            nc.tensor.matmul(out=pn[:], lhsT=vT[:], rhs=w3v[:], start=True, stop=True)
            x3b = sb.tile([128, 128], f32, tag="h0t")
            nc.sync.dma_start(out=x3b[:], in_=XW3[b])
            t1 = sb.tile([128, 128], f32, tag="t1")
            nc.vector.tensor_add(out=t1[:], in0=x3b[:], in1=pn[:])
            na = sb.tile([128, 128], f32, tag="h")
            nc.vector.tensor_relu(out=na[:], in_=t1[:])
            gb = sb.tile([128, NG], f32, tag="gb")
            nc.vector.tensor_scalar(
                out=gb[:], in0=iota5[:], scalar1=bidx[:, b:b + 1],
                scalar2=None, op0=mybir.AluOpType.is_equal)
            for g4 in range(4):
                po = ps.tile([128, 128], f32, tag="psw", name="po")
                nc.tensor.matmul(out=po[:], lhsT=gb[:, g4 * 128:(g4 + 1) * 128],
                                 rhs=na[:], start=True, stop=True)
                gsl = slice(g4 * 128, (g4 + 1) * 128)
                nc.vector.tensor_add(out=out_acc[:, gsl], in0=out_acc[:, gsl], in1=po[:])
        for g4 in range(4):
            nc.scalar.dma_start(out=outp[g4 * 128:(g4 + 1) * 128, :],
                                in_=out_acc[:, g4 * 128:(g4 + 1) * 128])

    nc.compile()
    _prog = nc
    return nc


def _host_layout(x, edge_attr, edge_index, batch):
    src_all = np.asarray(edge_index[0]).astype(np.int64)
    dst_all = np.asarray(edge_index[1]).astype(np.int64)
    batch = np.asarray(batch).astype(np.int64)

    xTfull = np.zeros((133, NPAD), np.float32)
    xTfull[:, :N] = np.asarray(x, np.float32).T
    batch_pad = np.zeros(NPAD, np.int64)
    batch_pad[:N] = batch

    per_core = []
    for c in range(NC):
        lo = c * ELOC
        src = src_all[lo:lo + ELOC]
        dst = dst_all[lo:lo + ELOC]
        order = np.argsort(dst, kind="stable")
        dsts = dst[order]
        blk = dsts >> 7
        cnt = np.bincount(blk, minlength=NBLK)
        assert cnt.max() <= CBLK, f"block overflow {cnt.max()}"
        start = np.zeros(NBLK, np.int64)
        start[1:] = np.cumsum(cnt)[:-1]
        rank = np.arange(ELOC) - start[blk]
        pos_sorted = blk * CBLK + rank
        posmap = np.empty(ELOC, np.int64)
        posmap[order] = pos_sorted

        src_pad = np.full(EPAD, NPAD - 1, np.int32)
        src_pad[pos_sorted] = src[order].astype(np.int32)
        rev_pad = np.arange(EPAD, dtype=np.int32)
        rev_pad[posmap] = posmap[np.arange(ELOC) ^ 1].astype(np.int32)
        dst_low = np.zeros(EPAD, np.float32)
        dst_low[pos_sorted] = (dsts & 127).astype(np.float32)

        eaTc = np.zeros((14, EPAD), np.float32)
        eaTc[:, pos_sorted] = np.asarray(edge_attr[lo:lo + ELOC], np.float32)[order].T

        nlo = c * NSLICE * 128
        batc = batch_pad[nlo:nlo + NSLICE * 128].astype(np.float32)

        per_core.append(dict(
            eaT=np.ascontiguousarray(eaTc),
            srcT=np.ascontiguousarray(src_pad.reshape(T, 128).T),
            revT=np.ascontiguousarray(rev_pad.reshape(T, 128).T),
            dstT=np.ascontiguousarray(dst_low.reshape(T, 128).T),
            batT=np.ascontiguousarray(batc.reshape(NSLICE, 128).T),
            xsT=np.ascontiguousarray(xTfull[:, nlo:nlo + NSLICE * 128]),
        ))
    return per_core


def kernel(x, edge_attr, W1, W2, W3, edge_index, rev_index, batch):
    global LAST_EXEC_NS
    from concourse.bass_utils import run_bass_kernel_spmd

    x = np.asarray(x, np.float32)
    edge_attr = np.asarray(edge_attr, np.float32)
    W1 = np.asarray(W1, np.float32)
    W2m = np.asarray(W2, np.float32)
    W3 = np.asarray(W3, np.float32)

    nc = _build_program()
    per_core = _host_layout(x, edge_attr, edge_index, batch)

    shared = dict(
        W1x1=np.ascontiguousarray(W1[0:128]),
        W1x2=np.ascontiguousarray(W1[128:133]),
        W1e=np.ascontiguousarray(W1[133:147]),
        W2=W2m,
        W3x1=np.ascontiguousarray(W3[0:128]),
        W3x2=np.ascontiguousarray(W3[128:133]),
        W3v=np.ascontiguousarray(W3[133:261]),
    )
    in_maps = [{**shared, **pc} for pc in per_core]

    trace = os.environ.get("BASS_KERNEL_TRACE", "0") == "1"
    import time as _time
    t0 = _time.time()
    res = run_bass_kernel_spmd(nc, in_maps, list(range(NC)), trace=trace)
    t1 = _time.time()
    LAST_EXEC_NS = res.exec_time_ns
    if LAST_EXEC_NS is None:
        LAST_EXEC_NS = int((t1 - t0) * 1e9)  # wall-clock fallback (incl. upload)

    out = np.zeros((NG, H), np.float32)
    for c in range(NC):
        out += res.results[c]["outp"]
    return out


# revision 8
# speedup vs baseline: 6.1113x; 2.5485x over previous
"""DMPNN encoder on 8 Trainium2 NeuronCores (Bass/Tile, SPMD).

Strategy: shard undirected edge pairs across cores (reverse edges stay
local). Each core sorts its edges by dst into a padded layout (392
node-blocks x 3 tiles of 128 edges). Message passing keeps W2 on the
edge side (consumer form): per block,
  m = node_in[src] - h_prev[rev]   (indirect row gathers, wide sub)
  h = relu(h0 + m @ W2)            (per-tile transpose + matmul)
Segment-sum is a one-hot matmul per tile; one-hots are built on device
with a broadcast is_equal against an iota, so no one-hot uploads.
ea @ W1e for a whole block is a single matmul against a block-diagonal
[42,384] weight. Node features are uploaded as per-core slices only;
x@W1x is AllGathered; per-depth node aggregates are ReduceScattered and
AllGathered. Data plane is bf16 (PSUM accumulation in f32).
"""
import sys, os
sys.path.insert(0, "/opt/trn_rl_repo")
import numpy as np

N = 50000
E = 800000
H = 128
NC = 8
ELOC = E // NC            # 100000
NBLK = 392
NPAD = NBLK * 128         # 50176
TPB = 3
CBLK = TPB * 128          # 384
T = NBLK * TPB            # 1176 tiles of 128 edges
EPAD = T * 128            # 150528
NSLICE = NBLK // NC       # 49 blocks per core slice
NG = 512

_prog = None
LAST_EXEC_NS = None


def _build_program():
    global _prog
    if _prog is not None:
        return _prog
    import concourse.bass as bass
    import concourse.mybir as mybir
    import concourse.tile as tile
    from concourse import bacc
    from concourse.masks import make_identity
    from contextlib import ExitStack

    f32 = mybir.dt.float32
    i32 = mybir.dt.int32
    bf16 = mybir.dt.bfloat16

    nc = bacc.Bacc("TRN2", target_bir_lowering=False, debug=False, num_devices=NC)

    def inp(name, shape, dt):
        return nc.dram_tensor(name, shape, dt, kind="ExternalInput").ap()

    xsT  = inp("xsT",  [133, NSLICE * 128], bf16)
    eaT3 = inp("eaT3", [3 * 14, NBLK * 128], bf16)
    W1x1 = inp("W1x1", [128, 128], bf16)
    W1x2 = inp("W1x2", [5, 128], bf16)
    W1e3 = inp("W1e3", [3 * 14, CBLK], bf16)
    W2   = inp("W2",   [128, 128], bf16)
    W3x1 = inp("W3x1", [128, 128], bf16)
    W3x2 = inp("W3x2", [5, 128], bf16)
    W3v  = inp("W3v",  [128, 128], bf16)
    srcT = inp("srcT", [128, T], i32)
    revT = inp("revT", [128, T], i32)
    dstT = inp("dstT", [128, T], f32)
    batT = inp("batT", [128, NSLICE], f32)
    outp = nc.dram_tensor("outp", [NG, H], f32, kind="ExternalOutput").ap()

    XWs = nc.dram_tensor("XWs", [NSLICE, 128, H], bf16).ap()
    XWf = nc.dram_tensor("XWf", [NBLK, 128, H], bf16, addr_space="Shared").ap()
    XW3 = nc.dram_tensor("XW3", [NSLICE, 128, H], bf16).ap()
    H0  = nc.dram_tensor("H0",  [NBLK, 128, CBLK], bf16).ap()
    H1  = nc.dram_tensor("H1",  [NBLK, 128, CBLK], bf16).ap()
    ndin = nc.dram_tensor("ndin", [NBLK, 128, H], bf16).ap()
    nsl  = nc.dram_tensor("nsl",  [NSLICE, 128, H], bf16).ap()
    NIf  = nc.dram_tensor("NIf",  [NBLK, 128, H], bf16, addr_space="Shared").ap()

    XWf_rows = XWf.rearrange("a p c -> (a p) c")
    NIf_rows = NIf.rearrange("a p c -> (a p) c")
    H0_rows = H0.rearrange("a p (j c) -> (a p j) c", c=128)
    H1_rows = H1.rearrange("a p (j c) -> (a p j) c", c=128)

    groups = [list(range(NC))]

    with tile.TileContext(nc) as tc, ExitStack() as ctx:
        consts = ctx.enter_context(tc.tile_pool(name="consts", bufs=1))
        sb = ctx.enter_context(tc.tile_pool(name="sb", bufs=3))
        ps = ctx.enter_context(tc.tile_pool(name="ps", bufs=2, space="PSUM"))

        identb = consts.tile([128, 128], bf16, name="identb", tag="identb")
        make_identity(nc, identb[:])
        iota3 = consts.tile([128, CBLK], f32, name="iota3", tag="iota3")
        nc.gpsimd.iota(iota3[:], pattern=[[0, TPB], [1, 128]], base=0,
                       channel_multiplier=0, allow_small_or_imprecise_dtypes=True)
        iota5 = consts.tile([128, NG], f32, name="iota5", tag="iota5")
        nc.gpsimd.iota(iota5[:], pattern=[[1, NG]], base=0, channel_multiplier=0,
                       allow_small_or_imprecise_dtypes=True)

        def const_tile(src_ap, shape, cname, dt):
            t_ = consts.tile(shape, dt, name=cname, tag=cname)
            nc.sync.dma_start(out=t_[:], in_=src_ap[:])
            return t_

        w1x1 = const_tile(W1x1, [128, 128], "w1x1", bf16)
        w1x2 = const_tile(W1x2, [5, 128], "w1x2", bf16)
        w1e3 = const_tile(W1e3, [42, CBLK], "w1e3", bf16)
        w2   = const_tile(W2,   [128, 128], "w2", bf16)
        w3x1 = const_tile(W3x1, [128, 128], "w3x1", bf16)
        w3x2 = const_tile(W3x2, [5, 128], "w3x2", bf16)
        w3v  = const_tile(W3v,  [128, 128], "w3v", bf16)
        sidx = const_tile(srcT, [128, T], "sidx", i32)
        ridx = const_tile(revT, [128, T], "ridx", i32)
        didx = const_tile(dstT, [128, T], "didx", f32)
        bidx = const_tile(batT, [128, NSLICE], "bidx", f32)

        iota3v = iota3[:].rearrange("p (a b) -> p a b", a=TPB)

        # ---- PRE: XWs = x_slice @ W1x, XW3 = x_slice @ W3x; AllGather XWs
        for b in range(NSLICE):
            cols = slice(b * 128, (b + 1) * 128)
            xt1 = sb.tile([128, 128], bf16, tag="xt1")
            nc.sync.dma_start(out=xt1[:], in_=xsT[0:128, cols])
            xt2 = sb.tile([5, 128], bf16, tag="xt2")
            nc.sync.dma_start(out=xt2[:], in_=xsT[128:133, cols])
            pw = ps.tile([128, 128], f32, tag="pn")
            nc.tensor.matmul(out=pw[:], lhsT=xt1[:], rhs=w1x1[:], start=True, stop=False)
            nc.tensor.matmul(out=pw[:], lhsT=xt2[:], rhs=w1x2[:], start=False, stop=True)
            xwb = sb.tile([128, 128], bf16, tag="xwb")
            nc.vector.tensor_copy(out=xwb[:], in_=pw[:])
            nc.scalar.dma_start(out=XWs[b], in_=xwb[:])
            pw3 = ps.tile([128, 128], f32, tag="pn")
            nc.tensor.matmul(out=pw3[:], lhsT=xt1[:], rhs=w3x1[:], start=True, stop=False)
            nc.tensor.matmul(out=pw3[:], lhsT=xt2[:], rhs=w3x2[:], start=False, stop=True)
            xwb3 = sb.tile([128, 128], bf16, tag="xwb")
            nc.vector.tensor_copy(out=xwb3[:], in_=pw3[:])
            nc.scalar.dma_start(out=XW3[b], in_=xwb3[:])
        nc.gpsimd.collective_compute(
            "AllGather", mybir.AluOpType.bypass, replica_groups=groups,
            ins=[XWs[:]], outs=[XWf[:]])

        # ---- edge sweeps
        def sweep(k):
            h_rd_rows = H0_rows if k == 2 else H1_rows
            for b in range(NBLK):
                t0 = b * TPB
                gw = sb.tile([128, CBLK], bf16, tag="gw")
                if k == 1:
                    for j in range(TPB):
                        nc.gpsimd.indirect_dma_start(
                            out=gw[:, j * 128:(j + 1) * 128], out_offset=None,
                            in_=XWf_rows,
                            in_offset=bass.IndirectOffsetOnAxis(
                                ap=sidx[:, t0 + j:t0 + j + 1], axis=0))
                    e3 = sb.tile([42, 128], bf16, tag="e3")
                    nc.sync.dma_start(out=e3[:], in_=eaT3[:, b * 128:(b + 1) * 128])
                    pz = ps.tile([128, CBLK], f32, tag="pz")
                    nc.tensor.matmul(out=pz[:], lhsT=e3[:], rhs=w1e3[:],
                                     start=True, stop=True)
                else:
                    for j in range(TPB):
                        nc.gpsimd.indirect_dma_start(
                            out=gw[:, j * 128:(j + 1) * 128], out_offset=None,
                            in_=NIf_rows,
                            in_offset=bass.IndirectOffsetOnAxis(
                                ap=sidx[:, t0 + j:t0 + j + 1], axis=0))
                    rw = sb.tile([128, CBLK], bf16, tag="rw")
                    for j in range(TPB):
                        nc.gpsimd.indirect_dma_start(
                            out=rw[:, j * 128:(j + 1) * 128], out_offset=None,
                            in_=h_rd_rows,
                            in_offset=bass.IndirectOffsetOnAxis(
                                ap=ridx[:, t0 + j:t0 + j + 1], axis=0))
                    mw = sb.tile([128, CBLK], bf16, tag="mw")
                    nc.vector.tensor_sub(out=mw[:], in0=gw[:], in1=rw[:])
                    h0w = sb.tile([128, CBLK], bf16, tag="h0w")
                    nc.sync.dma_start(out=h0w[:], in_=H0[b])
                    pz = ps.tile([128, CBLK], f32, tag="pz")
                    for j in range(TPB):
                        sl = slice(j * 128, (j + 1) * 128)
                        pT = ps.tile([128, 128], bf16, tag="pT")
                        nc.tensor.transpose(out=pT[:], in_=mw[:, sl], identity=identb[:])
                        mT = sb.tile([128, 128], bf16, tag="mT")
                        nc.vector.tensor_copy(out=mT[:], in_=pT[:])
                        nc.tensor.matmul(out=pz[:, sl], lhsT=mT[:], rhs=w2[:],
                                         start=True, stop=True)
                tw = sb.tile([128, CBLK], bf16, tag="tw")
                if k == 1:
                    nc.vector.tensor_add(out=tw[:], in0=pz[:], in1=gw[:])
                else:
                    nc.vector.tensor_add(out=tw[:], in0=pz[:], in1=h0w[:])
                hw = sb.tile([128, CBLK], bf16, tag="hw")
                nc.vector.tensor_relu(out=hw[:], in_=tw[:])
                if k == 1:
                    nc.scalar.dma_start(out=H0[b], in_=hw[:])
                elif k == 2:
                    nc.scalar.dma_start(out=H1[b], in_=hw[:])
                st = sb.tile([128, CBLK], bf16, tag="st")
                nc.vector.tensor_tensor(
                    out=st[:].rearrange("p (a b) -> p a b", a=TPB),
                    in0=iota3v,
                    in1=didx[:, t0:t0 + TPB].unsqueeze(2).broadcast_to([128, TPB, 128]),
                    op=mybir.AluOpType.is_equal)
                pnode = ps.tile([128, 128], f32, tag="pn")
                for j in range(TPB):
                    sl = slice(j * 128, (j + 1) * 128)
                    nc.tensor.matmul(out=pnode[:], lhsT=st[:, sl], rhs=hw[:, sl],
                                     start=(j == 0), stop=(j == TPB - 1))
                nb = sb.tile([128, 128], bf16, tag="nb")
                nc.vector.tensor_copy(out=nb[:], in_=pnode[:])
                nc.scalar.dma_start(out=ndin[b], in_=nb[:])

        def collective(k):
            nc.gpsimd.collective_compute(
                "ReduceScatter", mybir.AluOpType.add, replica_groups=groups,
                ins=[ndin[:]], outs=[nsl[:]])
            if k < 3:
                nc.gpsimd.collective_compute(
                    "AllGather", mybir.AluOpType.bypass, replica_groups=groups,
                    ins=[nsl[:]], outs=[NIf[:]])

        sweep(1)
        collective(1)
        sweep(2)
        collective(2)
        sweep(3)
        collective(3)

        # ---- final: node_attr = relu(XW3 + vmsg @ W3v); out += GB^T @ node_attr
        out_acc = consts.tile([128, NG], f32, name="out_acc", tag="out_acc")
        nc.vector.memset(out_acc[:], 0.0)
        for b in range(NSLICE):
            vb = sb.tile([128, 128], bf16, tag="vb")
            nc.sync.dma_start(out=vb[:], in_=nsl[b])
            pT = ps.tile([128, 128], bf16, tag="pT")
            nc.tensor.transpose(out=pT[:], in_=vb[:], identity=identb[:])
            vT = sb.tile([128, 128], bf16, tag="mT")
            nc.vector.tensor_copy(out=vT[:], in_=pT[:])
            pn = ps.tile([128, 128], f32, tag="pn")
            nc.tensor.matmul(out=pn[:], lhsT=vT[:], rhs=w3v[:], start=True, stop=True)
            x3b = sb.tile([128, 128], bf16, tag="vb")
            nc.sync.dma_start(out=x3b[:], in_=XW3[b])
            tn = sb.tile([128, 128], bf16, tag="tn")
            nc.vector.tensor_add(out=tn[:], in0=pn[:], in1=x3b[:])
            na = sb.tile([128, 128], bf16, tag="na")
            nc.vector.tensor_relu(out=na[:], in_=tn[:])
            gb = sb.tile([128, NG], bf16, tag="gb")
            nc.vector.tensor_scalar(
                out=gb[:], in0=iota5[:], scalar1=bidx[:, b:b + 1],
                scalar2=None, op0=mybir.AluOpType.is_equal)
            for g4 in range(4):
                gsl = slice(g4 * 128, (g4 + 1) * 128)
                po = ps.tile([128, 128], f32, tag="pn", name="po")
                nc.tensor.matmul(out=po[:], lhsT=gb[:, gsl], rhs=na[:],
                                 start=True, stop=True)
                nc.vector.tensor_add(out=out_acc[:, gsl], in0=out_acc[:, gsl],
                                     in1=po[:])
        for g4 in range(4):
            nc.scalar.dma_start(out=outp[g4 * 128:(g4 + 1) * 128, :],
                                in_=out_acc[:, g4 * 128:(g4 + 1) * 128])

    nc.compile()
    _prog = nc
    return nc


def _host_layout(x, edge_attr, edge_index, batch):
    import ml_dtypes
    bf = ml_dtypes.bfloat16
    src_all = np.asarray(edge_index[0]).astype(np.int64)
    dst_all = np.asarray(edge_index[1]).astype(np.int64)
    batch = np.asarray(batch).astype(np.int64)

    xTfull = np.zeros((133, NPAD), np.float32)
    xTfull[:, :N] = np.asarray(x, np.float32).T
    xTfull = xTfull.astype(bf)
    batch_pad = np.zeros(NPAD, np.int64)
    batch_pad[:N] = batch

    # row index into the wide [NBLK,128,CBLK] h-state, viewed as rows of 128:
    # slot s=(b,j,p) -> row b*CBLK + p*TPB + j
    def h_row(s):
        return (s // CBLK) * CBLK + (s % 128) * TPB + (s % CBLK) // 128

    per_core = []
    for c in range(NC):
        lo = c * ELOC
        src = src_all[lo:lo + ELOC]
        dst = dst_all[lo:lo + ELOC]
        order = np.argsort(dst, kind="stable")
        dsts = dst[order]
        blk = dsts >> 7
        cnt = np.bincount(blk, minlength=NBLK)
        assert cnt.max() <= CBLK, f"block overflow {cnt.max()}"
        start = np.zeros(NBLK, np.int64)
        start[1:] = np.cumsum(cnt)[:-1]
        rank = np.arange(ELOC) - start[blk]
        pos_sorted = blk * CBLK + rank
        posmap = np.empty(ELOC, np.int64)
        posmap[order] = pos_sorted

        src_pad = np.full(EPAD, NPAD - 1, np.int32)
        src_pad[pos_sorted] = src[order].astype(np.int32)
        dst_low = np.zeros(EPAD, np.float32)
        dst_low[pos_sorted] = (dsts & 127).astype(np.float32)

        s_all = np.arange(EPAD, dtype=np.int64)
        rev_rows = h_row(s_all).astype(np.int32)
        rev_rows[posmap] = h_row(posmap[np.arange(ELOC) ^ 1]).astype(np.int32)

        # eaT3[14*j + f, b*128 + p] = ea_sorted[slot][f]
        ea_sorted = np.asarray(edge_attr[lo:lo + ELOC], np.float32)[order]
        eaT3 = np.zeros((42, NBLK * 128), bf)
        s = pos_sorted
        col = (s // CBLK) * 128 + (s % 128)
        row0 = ((s % CBLK) // 128) * 14
        eaT3[row0[:, None] + np.arange(14)[None, :], col[:, None]] = ea_sorted.astype(bf)

        nlo = c * NSLICE * 128
        batc = batch_pad[nlo:nlo + NSLICE * 128].astype(np.float32)

        per_core.append(dict(
            eaT3=eaT3,
            srcT=np.ascontiguousarray(src_pad.reshape(T, 128).T),
            revT=np.ascontiguousarray(rev_rows.reshape(T, 128).T),
            dstT=np.ascontiguousarray(dst_low.reshape(T, 128).T),
            batT=np.ascontiguousarray(batc.reshape(NSLICE, 128).T),
            xsT=np.ascontiguousarray(xTfull[:, nlo:nlo + NSLICE * 128]),
        ))
    return per_core


def kernel(x, edge_attr, W1, W2, W3, edge_index, rev_index, batch):
    global LAST_EXEC_NS
    import ml_dtypes
    from concourse.bass_utils import run_bass_kernel_spmd
    bf = ml_dtypes.bfloat16

    x = np.asarray(x, np.float32)
    edge_attr = np.asarray(edge_attr, np.float32)
    W1 = np.asarray(W1, np.float32)
    W2m = np.asarray(W2, np.float32)
    W3 = np.asarray(W3, np.float32)

    nc = _build_program()
    per_core = _host_layout(x, edge_attr, edge_index, batch)

    w1e = W1[133:147]                      # [14, 128]
    w1e3 = np.zeros((42, CBLK), np.float32)
    for j in range(TPB):
        w1e3[14 * j:14 * (j + 1), 128 * j:128 * (j + 1)] = w1e

    shared = dict(
        W1x1=np.ascontiguousarray(W1[0:128]).astype(bf),
        W1x2=np.ascontiguousarray(W1[128:133]).astype(bf),
        W1e3=w1e3.astype(bf),
        W2=W2m.astype(bf),
        W3x1=np.ascontiguousarray(W3[0:128]).astype(bf),
        W3x2=np.ascontiguousarray(W3[128:133]).astype(bf),
        W3v=np.ascontiguousarray(W3[133:261]).astype(bf),
    )
    in_maps = [{**shared, **pc} for pc in per_core]

    trace = os.environ.get("BASS_KERNEL_TRACE", "0") == "1"
    import time as _time
    t0 = _time.time()
    res = run_bass_kernel_spmd(nc, in_maps, list(range(NC)), trace=trace)
    t1 = _time.time()
    LAST_EXEC_NS = res.exec_time_ns
    if LAST_EXEC_NS is None:
        LAST_EXEC_NS = int((t1 - t0) * 1e9)  # wall-clock fallback (incl. upload)

    out = np.zeros((NG, H), np.float32)
    for c in range(NC):
        out += res.results[c]["outp"]
    return out
